# revision 20
# baseline (speedup 1.0000x reference)
"""Trainium2 Bass kernel for nn_MHA_42391327211690.

MHA: B=1, S=2048, E=2048, H=32 q-heads, HKV=8 kv-heads, D=64, RoPE(rot=64,
GPT-NeoX style) on q/k, causal GQA attention, out-projection with bias.

Distribution (8 NeuronCores, tensor-parallel by heads):
  - core i computes q-heads 4i..4i+3 and kv-head i (Wqkv column-sharded),
  - attention entirely local (GQA groups align with the shard),
  - AllToAll redistributes ctx^T from head-sharded to sequence-sharded,
  - out-projection computed per-core for its 256-row sequence slice
    (weights replicated), host concatenates the slices.

All matmuls run as float32r (FP22 mantissa-truncated fp32, full PE rate).
Scores layout is transposed ([t, sq]) so softmax normalization needs no
transposes: denominator comes from an appended ones-column in V, applied
as a reciprocal multiply on the ctx^T tile.
"""

from contextlib import ExitStack

import numpy as np

import concourse.bass as bass
import concourse.bacc as bacc
import concourse.tile as tile
from concourse import mybir
from concourse.bass_utils import run_bass_kernel_spmd

F32 = mybir.dt.float32
F32R = mybir.dt.float32r
AF = mybir.ActivationFunctionType
ALU = mybir.AluOpType

B, E = 1, 2048
H, HKV, D = 32, 8, 64
ROT, BASE = 64, 10000.0
NCORES = 8
HL = H // NCORES            # 4 local q heads
OPL = (HL + 2) * D          # 384 local qkv output rows (q | k | v)
SCALE = float(D) ** -0.5


def _r(x):
    return x.bitcast(F32R)


def build_nc(S=2048, n_cores=NCORES):
    """Build the SPMD Bass program (identical on every core)."""
    SEG = S // n_cores      # per-core output sequence slice
    NT = S // 128           # t-blocks (key blocks)
    NG = S // 512           # sq groups of 512
    NE = E // 128           # contraction tiles for qkv / out proj
    FD = HL * D             # 256 local ctx rows

    nc = bacc.Bacc("TRN2", target_bir_lowering=False, debug=False,
                   num_devices=n_cores)

    xT = nc.dram_tensor("xT", [E, S], F32, kind="ExternalInput")
    wqkvT = nc.dram_tensor("wqkvT", [E, OPL], F32, kind="ExternalInput")
    bqkv = nc.dram_tensor("bqkv", [OPL, 1], F32, kind="ExternalInput")
    cc_d = nc.dram_tensor("cc", [128, S], F32, kind="ExternalInput")
    ss_d = nc.dram_tensor("ss", [128, S], F32, kind="ExternalInput")
    triu_d = nc.dram_tensor("triu", [128, 128], F32, kind="ExternalInput")
    id_d = nc.dram_tensor("ident", [128, 64], F32, kind="ExternalInput")
    woT = nc.dram_tensor("woT", [E, E], F32, kind="ExternalInput")
    outb_d = nc.dram_tensor("outb", [128, E], F32, kind="ExternalInput")
    outS = nc.dram_tensor("outS", [SEG, E], F32, kind="ExternalOutput")

    with tile.TileContext(nc) as tc, ExitStack() as ctx:
        outc = ctx.enter_context(tc.tile_pool(name="outc", bufs=1))
        outb = outc.tile([128, E], F32)
        nc.sync.dma_start(outb[:], outb_d[:])
        ctx_pool = ctx.enter_context(tc.tile_pool(name="ctxsb", bufs=1))
        ctx_sb = [ctx_pool.tile([64, S], F32, tag=f"c{i}", name=f"ctxsb{i}") for i in range(HL)]
        wo_sb = {}

        ab = ExitStack()
        consts = ab.enter_context(tc.tile_pool(name="consts", bufs=1))
        cc = consts.tile([128, S], F32)
        ss = consts.tile([128, S], F32)
        triu = consts.tile([128, 128], F32)
        ident = consts.tile([128, 64], F32)
        bq = [consts.tile([128, 1], F32, tag=f"bq{j}", name=f"bq{j}") for j in range(3)]
        nc.sync.dma_start(cc[:], cc_d[:])
        nc.sync.dma_start(ss[:], ss_d[:])
        nc.sync.dma_start(triu[:], triu_d[:])
        nc.sync.dma_start(ident[:], id_d[:])
        for j in range(3):
            nc.sync.dma_start(bq[j][:], bqkv[j * 128:(j + 1) * 128, :])

        # persistent qkv activations (phases A+B)
        qkv_pool = ab.enter_context(tc.tile_pool(name="qkv", bufs=1))
        q_sb = [qkv_pool.tile([128, S], F32, tag=f"q{i}", name=f"qsb{i}") for i in range(HL // 2)]
        kv_sb = qkv_pool.tile([128, S], F32, tag="kv")      # k rows 0:64, v rows 64:128
        kdup = qkv_pool.tile([128, S], F32, tag="kdup")     # roped k duplicated
        v_pool = ab.enter_context(tc.tile_pool(name="vsb", bufs=1))
        v_sb = [v_pool.tile([128, D + 1], F32R, tag=f"v{t}", name=f"vsb{t}") for t in range(NT)]

        # ---------------- Phase A: QKV + RoPE + V transpose -------------
        with tc.tile_pool(name="xw", bufs=1) as xw_pool, \
             tc.tile_pool(name="ropet", bufs=2) as rope_pool, \
             tc.tile_pool(name="psqkv", bufs=2, space="PSUM") as ps_qkv, \
             tc.tile_pool(name="psvt", bufs=2, space="PSUM") as ps_vt:

            wq_sb = [xw_pool.tile([128, OPL], F32R, tag=f"wq{e}", name=f"wqsb{e}") for e in range(NE)]
            for e in range(NE):
                nc.sync.dma_start(wq_sb[e][:], wqkvT[e * 128:(e + 1) * 128, :].bitcast(F32R))
            # x^T resident in halves of the contraction dim (64KB/part each)
            xh = [xw_pool.tile([128, S], F32R, tag=f"x{e}", bufs=1, name=f"xh{e}")
                  for e in range(NE // 2)]

            NSG = S // 512
            for eh in range(2):
                for e in range(NE // 2):
                    et = eh * (NE // 2) + e
                    nc.sync.dma_start(xh[e][:], xT[et * 128:(et + 1) * 128, :].bitcast(F32R))
                for sg in range(NSG):
                    sgs = slice(sg * 512, (sg + 1) * 512)
                    ps = ps_qkv.tile([128, 1536], F32, tag="qkvps")
                    for e in range(NE // 2):
                        et = eh * (NE // 2) + e
                        for j in range(3):
                            nc.tensor.matmul(
                                ps[:, j * 512:(j + 1) * 512],
                                _r(wq_sb[et][:, j * 128:(j + 1) * 128]),
                                _r(xh[e][:, sgs]),
                                start=(e == 0), stop=(e == NE // 2 - 1))
                    dsts = [q_sb[0], q_sb[1], kv_sb]
                    for j in range(3):
                        # q tiles are consumed by fp32r matmuls; every write
                        # to them must carry the fp32r rounding tag
                        out_ap = dsts[j][:, sgs]
                        if j < 2:
                            out_ap = out_ap.bitcast(F32R)
                        if eh == 0:
                            nc.scalar.activation(
                                out_ap, ps[:, j * 512:(j + 1) * 512],
                                AF.Identity, bias=bq[j][:], scale=1.0)
                        else:
                            nc.vector.scalar_tensor_tensor(
                                out_ap, ps[:, j * 512:(j + 1) * 512],
                                1.0, dsts[j][:, sgs], ALU.mult, ALU.add)
                    if eh == 1:
                        # ---- RoPE: swapped halves built via SBUF->SBUF DMA
                        # (DVE requires equal base partitions on both inputs)
                        for qi in range(HL // 2):
                            qt = q_sb[qi]
                            qs = rope_pool.tile([128, 512], F32, tag="qs")
                            for b in range(4):
                                nc.sync.dma_start(
                                    qs[b * 32:(b + 1) * 32, :],
                                    qt[(b ^ 1) * 32:((b ^ 1) + 1) * 32, sgs])
                            t1 = rope_pool.tile([128, 512], F32, tag="t1")
                            t2 = rope_pool.tile([128, 512], F32, tag="t2")
                            nc.vector.tensor_mul(t1[:], qt[:, sgs], cc[:, sgs])
                            nc.vector.tensor_mul(t2[:], qs[:], ss[:, sgs])
                            nc.vector.tensor_add(qt[:, sgs].bitcast(F32R), t1[:], t2[:])
                        # k: rows 0:64 of kv_sb -> roped into kdup, then dup'd
                        ks = rope_pool.tile([64, 512], F32, tag="ks")
                        for b in range(2):
                            nc.sync.dma_start(
                                ks[b * 32:(b + 1) * 32, :],
                                kv_sb[(b ^ 1) * 32:((b ^ 1) + 1) * 32, sgs])
                        t1 = rope_pool.tile([64, 512], F32, tag="kt1")
                        t2 = rope_pool.tile([64, 512], F32, tag="kt2")
                        nc.vector.tensor_mul(t1[:], kv_sb[0:64, sgs], cc[0:64, sgs])
                        nc.vector.tensor_mul(t2[:], ks[:], ss[0:64, sgs])
                        nc.vector.tensor_add(kdup[0:64, sgs].bitcast(F32R), t1[:], t2[:])
                        nc.sync.dma_start(kdup[64:128, sgs].bitcast(F32R),
                                          kdup[0:64, sgs].bitcast(F32R))
                        # ---- V transpose: kv_sb rows 64:128 -> v_sb tiles ----
                        for c in range(4):
                            t = sg * 4 + c
                            pvt = ps_vt.tile([128, 64], F32, tag="vt")
                            nc.tensor.transpose(
                                pvt[:],
                                kv_sb[64:128, sg * 512 + c * 128: sg * 512 + (c + 1) * 128],
                                ident[64:128, :])
                            nc.vector.memset(v_sb[t][:, 64:65].bitcast(F32), 1.0)
                            nc.vector.tensor_copy(v_sb[t][:, 0:64], pvt[:])

        # output-projection weights pool (created after the x/wqkv pool is
        # released so its SBUF region does not overlap phase A's)
        wo_pool = ctx.enter_context(tc.tile_pool(name="wo", bufs=1, side="right"))
        # prefetch first half of wo during attention
        for f in range(NE):
            wo_sb[(f, 0)] = wo_pool.tile([128, 1024], F32R, tag=f"wo{f}", name=f"woA{f}")
            nc.sync.dma_start(wo_sb[(f, 0)][:], woT[f * 128:(f + 1) * 128, 0:1024].bitcast(F32R))

        # ---------------- Phase B: attention --------------------------
        with tc.tile_pool(name="exps", bufs=3) as exps_pool, \
             tc.tile_pool(name="rcp", bufs=2) as rcp_pool, \
             tc.tile_pool(name="pss", bufs=2, space="PSUM") as ps_s, \
             tc.tile_pool(name="psctx", bufs=2, space="PSUM") as ps_ctx:
            ones64 = rcp_pool.tile([1, 64], F32R, tag="ones", bufs=1)
            nc.vector.memset(ones64[:].bitcast(F32), 1.0)
            for hp in range(HL // 2):
                qt = q_sb[hp]
                for g in range(NG):
                    gs = slice(g * 512, (g + 1) * 512)
                    pc_e = ps_ctx.tile([D + 1, 512], F32, tag="ctx_e")
                    pc_o = ps_ctx.tile([D + 1, 512], F32, tag="ctx_o")
                    ntb = 4 * g + 4
                    for t in range(ntb):
                        ts_ = slice(t * 128, (t + 1) * 128)
                        pss = ps_s.tile([128, 1024], F32, tag="s")
                        nc.tensor.matmul(pss[:, 0:512], _r(kdup[0:64, ts_]),
                                         _r(qt[0:64, gs]), start=True, stop=True)
                        nc.tensor.matmul(pss[:, 512:1024], _r(kdup[64:128, ts_]),
                                         _r(qt[64:128, gs]), start=True, stop=True)
                        ex = exps_pool.tile([128, 1024], F32R, tag="e")
                        nc.scalar.activation(ex[:], pss[:], AF.Exp, scale=SCALE)
                        j = t - 4 * g
                        if j > 0:
                            nc.vector.memset(ex[:, 0:j * 128].bitcast(F32), 0.0)
                            nc.vector.memset(ex[:, 512:512 + j * 128].bitcast(F32), 0.0)
                        if j >= 0:
                            for h2 in range(2):
                                sl = slice(h2 * 512 + j * 128, h2 * 512 + (j + 1) * 128)
                                nc.vector.tensor_mul(ex[:, sl], ex[:, sl], triu[:])
                        nc.tensor.matmul(pc_e[:], _r(v_sb[t][:]), _r(ex[:, 0:512]),
                                         start=(t == 0), stop=(t == ntb - 1))
                        nc.tensor.matmul(pc_o[:], _r(v_sb[t][:]), _r(ex[:, 512:1024]),
                                         start=(t == 0), stop=(t == ntb - 1))
                    rc_e = rcp_pool.tile([1, 512], F32R, tag="re")
                    rc_o = rcp_pool.tile([1, 512], F32R, tag="ro")
                    with nc.allow_low_precision(reason="fp32r matmul feed"):
                        nc.vector.reciprocal(rc_e[:], pc_e[64:65, :])
                        nc.vector.reciprocal(rc_o[:], pc_o[64:65, :])
                    bc = ps_s.tile([128, 1024], F32, tag="s")
                    nc.tensor.matmul(bc[0:64, 0:512], _r(ones64[:]), _r(rc_e[:]),
                                     start=True, stop=True)
                    nc.tensor.matmul(bc[0:64, 512:1024], _r(ones64[:]), _r(rc_o[:]),
                                     start=True, stop=True)
                    bc_sb = rcp_pool.tile([64, 1024], F32, tag="bcsb")
                    nc.vector.tensor_copy(bc_sb[:], bc[0:64, :])
                    nc.vector.tensor_mul(ctx_sb[2 * hp][:, gs], pc_e[0:64, :],
                                         bc_sb[:, 0:512])
                    nc.vector.tensor_mul(ctx_sb[2 * hp + 1][:, gs], pc_o[0:64, :],
                                         bc_sb[:, 512:1024])

        ab.close()  # release qkv/v/consts SBUF before the out-projection

        # ---------------- Phase C: AllToAll + out projection -----------
        FDT = n_cores * 2 * 128  # total ctx rows (= H*D when full size)
        with tc.tile_pool(name="dram", bufs=1, space="DRAM") as dram, \
             tc.tile_pool(name="cf", bufs=1) as cf_pool, \
             tc.tile_pool(name="osb", bufs=1) as out_pool, \
             tc.tile_pool(name="pso", bufs=2, space="PSUM") as ps_o:
            a2a_in = dram.tile([FDT, SEG], F32)
            a2a_out = dram.tile([FDT, SEG], F32)
            for j in range(n_cores):
                for h in range(HL):
                    nc.sync.dma_start(
                        a2a_in[j * 256 + h * 64: j * 256 + (h + 1) * 64, :],
                        ctx_sb[h][:, j * SEG:(j + 1) * SEG])
            nc.gpsimd.collective_compute(
                "AllToAll", ALU.bypass,
                replica_groups=[list(range(n_cores))],
                ins=[a2a_in[:]], outs=[a2a_out[:]])
            ctxF = [cf_pool.tile([128, SEG], F32R, tag=f"cf{f}", name=f"cfsb{f}") for f in range(NE)]
            for f in range(NE):
                nc.sync.dma_start(ctxF[f][:], a2a_out[f * 128:(f + 1) * 128, :].bitcast(F32R))
            out_sb = [out_pool.tile([128, E], F32, tag=f"ot{s}", name=f"osb{s}")
                      for s in range(SEG // 128)]
            for ehalf in range(2):
                if ehalf == 1:
                    for f in range(NE):
                        wo_sb[(f, 1)] = wo_pool.tile([128, 1024], F32R, tag=f"wo{f}", name=f"woB{f}")
                        nc.sync.dma_start(wo_sb[(f, 1)][:],
                                          woT[f * 128:(f + 1) * 128, 1024:2048].bitcast(F32R))
                for st in range(SEG // 128):
                    for egl in range(2):
                        eg = ehalf * 2 + egl
                        po = ps_o.tile([128, 512], F32, tag="o")
                        for f in range(NE):
                            nc.tensor.matmul(
                                po[:],
                                _r(ctxF[f][:, st * 128:(st + 1) * 128]),
                                _r(wo_sb[(f, ehalf)][:, egl * 512:(egl + 1) * 512]),
                                start=(f == 0), stop=(f == NE - 1))
                        nc.vector.scalar_tensor_tensor(
                            out_sb[st][:, eg * 512:(eg + 1) * 512], po[:], 1.0,
                            outb[:, eg * 512:(eg + 1) * 512],
                            ALU.mult, ALU.add)
            for st in range(SEG // 128):
                nc.sync.dma_start(outS[st * 128:(st + 1) * 128, :], out_sb[st][:])

    nc.compile()
    return nc


def shard_inputs(hidden_states, Wqkv_w, Wqkv_b, out_w, out_b, S=2048,
                 n_cores=NCORES):
    """Host-side sharding: returns per-core input maps."""
    x = np.asarray(hidden_states, np.float32).reshape(S, E)
    xT = np.ascontiguousarray(x.T)
    Wqkv_w = np.asarray(Wqkv_w, np.float32)
    Wqkv_b = np.asarray(Wqkv_b, np.float32)
    woT = np.ascontiguousarray(np.asarray(out_w, np.float32).T)
    outb = np.ascontiguousarray(np.broadcast_to(np.asarray(out_b, np.float32).reshape(1, E), (128, E)))

    inv = (1.0 / (BASE ** (np.arange(0, ROT, 2, dtype=np.float64) / ROT)))
    t = np.arange(S, dtype=np.float64)
    freqs = np.outer(t, inv)                      # [S, 32]
    cT = np.cos(freqs).T.astype(np.float32)       # [32, S]
    sT = np.sin(freqs).T.astype(np.float32)
    cc = np.tile(cT, (4, 1))                      # [128, S]
    ss = np.concatenate([-sT, sT, -sT, sT], axis=0)
    triu = (np.arange(128)[:, None] <= np.arange(128)[None, :]).astype(np.float32)
    ident = np.vstack([np.eye(64, dtype=np.float32)] * 2)

    in_maps = []
    for i in range(n_cores):
        hq = H // n_cores
        wq = Wqkv_w[i * hq * D:(i + 1) * hq * D]          # [256, E]
        wk = Wqkv_w[H * D + i * D: H * D + (i + 1) * D]   # [64, E]
        wv = Wqkv_w[(H + HKV) * D + i * D: (H + HKV) * D + (i + 1) * D]
        w_local = np.concatenate([wq, wk, wv], axis=0)    # [384, E]
        b_local = np.concatenate([
            Wqkv_b[i * hq * D:(i + 1) * hq * D],
            Wqkv_b[H * D + i * D: H * D + (i + 1) * D],
            Wqkv_b[(H + HKV) * D + i * D: (H + HKV) * D + (i + 1) * D]])
        in_maps.append({
            "xT": xT,
            "wqkvT": np.ascontiguousarray(w_local.T),
            "bqkv": np.ascontiguousarray(b_local.reshape(OPL, 1)),
            "cc": cc, "ss": ss, "triu": triu, "ident": ident,
            "woT": woT, "outb": outb,
        })
    return in_maps


def assemble(results, S=2048, n_cores=NCORES):
    out = np.concatenate([r["outS"] for r in results], axis=0)
    return out.reshape(B, S, E).astype(np.float32)


_NC_CACHE = {}


def _get_nc(S=2048):
    if S not in _NC_CACHE:
        _NC_CACHE[S] = build_nc(S=S)
    return _NC_CACHE[S]


def kernel(hidden_states, Wqkv_w, Wqkv_b, out_w, out_b, _trace=False):
    in_maps = shard_inputs(hidden_states, Wqkv_w, Wqkv_b, out_w, out_b)
    nc = _get_nc()
    res = run_bass_kernel_spmd(nc, in_maps, core_ids=list(range(NCORES)),
                               trace=_trace)
    out = assemble(res.results)
    if _trace:
        kernel.last_results = res
    return out
